# revision 4
# baseline (speedup 1.0000x reference)
"""Chamfer loss kernel for Trainium2, batch-parallel over 8 NeuronCores.

Per core (one batch element b):
  gts = src_points[b] @ R^T + t          (on device, bf16-pair matmul)
  P[i,j] = |gts_i|^2 + |recon_j|^2 - 2 gts_i . recon_j
  loss_b = sum_j min_i P + sum_i min_j P
Host sums the 8 partial losses.

Structure:
- Each distance tile comes out of a single augmented K=11 bf16 matmul:
  the -2*g.p cross terms use a bf16 hi/lo pair decomposition (hi*hi +
  hi*lo + lo*hi, ~2^-18 relative accuracy) and BOTH norms ride along as
  single-bf16 rows (k9: ones x yy, k10: xx x ones).  Norm errors are
  row/column-structured and mostly cancel in the summed loss.
- ACT stages each PSUM tile to bf16 SBUF (pure dtype copy, no bias).
- The staged bf16 tiles are reduced by DVE in 2x mode: two running
  elementwise-min accumulators for the per-column mins (even/odd blocks,
  so DVE isn't serialized on one dependency chain), and a batched binary
  min-tree (two row blocks at a time) for the per-row mins.
- Per-column mins are finished with PE transposes + free-axis folds, and
  everything is summed with a final ones-matmul across partitions.
"""

import os

# the axon client here has no NTFF profile hook; a stray BASS_TRACE=1 in the
# environment would crash run_bass_kernel_spmd on a missing import
os.environ["BASS_NEVER_TRACE"] = "1"

import ml_dtypes
import numpy as np

import concourse.bacc as bacc
import concourse.bass as bass
import concourse.mybir as mybir
import concourse.tile as tile
from concourse.bass_utils import run_bass_kernel_spmd

F32 = mybir.dt.float32
BF16 = mybir.dt.bfloat16
ALU = mybir.AluOpType
AX = mybir.AxisListType
AF = mybir.ActivationFunctionType

N_CORES = 8
NPTS = 4096          # points per set (both gts and recon)
NBLK = NPTS // 128   # 32 row blocks
HALF = 2048          # P tile free width (4 PSUM banks)

_CACHE = {}
LAST_RESULTS = None


def _build_kernel():
    nc = bacc.Bacc("TRN2", target_bir_lowering=False, debug=False)

    srcT = nc.declare_dram_parameter("srcT", [4, NPTS], F32, isOutput=False)
    reconT = nc.declare_dram_parameter("reconT", [4, NPTS], F32, isOutput=False)
    taug = nc.declare_dram_parameter("taug", [4, 4], F32, isOutput=False)
    ident = nc.declare_dram_parameter("ident", [128, 128], BF16, isOutput=False)
    cnorm = nc.declare_dram_parameter("cnorm", [8, 2], BF16, isOutput=False)
    cscal = nc.declare_dram_parameter("cscal", [8, 1], F32, isOutput=False)
    cones = nc.declare_dram_parameter("cones", [128, 1], F32, isOutput=False)
    loss = nc.declare_dram_parameter("loss", [1, 1], F32, isOutput=True)

    with tile.TileContext(nc) as tc:
        with tc.tile_pool(name="sb", bufs=1) as sb:
            prep_pool = tc.alloc_tile_pool(name="prep", bufs=1)
            # ---- phase 0: load inputs (chunked over DMA queues) ---------
            pts = prep_pool.tile([8, NPTS], F32) # rows 0-3 gts_aug, 4-7 recon_aug
            for c in range(4):
                cs = slice(c * 1024, (c + 1) * 1024)
                nc.sync.dma_start(out=pts[0:4, cs], in_=srcT[:, cs])
                nc.sync.dma_start(out=pts[4:8, cs], in_=reconT[:, cs])

            taug_sb = sb.tile([4, 4], F32)
            nc.sync.dma_start(out=taug_sb[:, :], in_=taug[:, :])
            ident_sb = sb.tile([128, 128], BF16)
            nc.sync.dma_start(out=ident_sb[:, :], in_=ident[:, :])
            norm_ones = sb.tile([8, 2], BF16)
            nc.sync.dma_start(out=norm_ones[:, :], in_=cnorm[:, :])
            scal = sb.tile([8, 1], F32)
            nc.sync.dma_start(out=scal[:, :], in_=cscal[:, :])
            ones128 = sb.tile([128, 1], F32)
            nc.sync.dma_start(out=ones128[:, :], in_=cones[:, :])

            # PE warm-up: ~40 tiny matmuls on the identity while inputs
            # load, so the transform/norm matmuls run at full PE clock
            with tc.tile_pool(name="warm_ps", bufs=1, space="PSUM") as wpp:
                warm_ps = wpp.tile([128, 128], F32)
                for _ in range(40):
                    nc.tensor.matmul(warm_ps[:, :], lhsT=ident_sb[:, :],
                                     rhs=ident_sb[:, :], start=True,
                                     stop=True)

            # ---- phase 1: operand prep ----------------------------------
            # bf16 hi/lo of the transform and of the source points
            th = sb.tile([4, 4], BF16)
            tl = sb.tile([4, 4], BF16)
            nc.vector.tensor_copy(th[:, :], taug_sb[:, :])
            nc.vector.scalar_tensor_tensor(tl[:, :], taug_sb[:, :], 1.0,
                                           th[:, :], ALU.mult, ALU.subtract)
            s_hi = prep_pool.tile([4, NPTS], BF16)
            s_lo = prep_pool.tile([4, NPTS], BF16)
            nc.vector.tensor_copy(s_hi[:, :], pts[0:4, :])
            nc.vector.scalar_tensor_tensor(s_lo[:, :], pts[0:4, :], 1.0,
                                           s_hi[:, :], ALU.mult, ALU.subtract)
            tlhs = sb.tile([12, 4], BF16)
            nc.sync.dma_start(out=tlhs[0:4, :], in_=th[:, :])
            nc.sync.dma_start(out=tlhs[4:8, :], in_=th[:, :])
            nc.sync.dma_start(out=tlhs[8:12, :], in_=tl[:, :])
            trhs = prep_pool.tile([12, NPTS], BF16)
            nc.sync.dma_start(out=trhs[0:4, :], in_=s_hi[:, :])
            nc.sync.dma_start(out=trhs[4:8, :], in_=s_lo[:, :])
            nc.sync.dma_start(out=trhs[8:12, :], in_=s_hi[:, :])

            # squares in single bf16: the staged distance tiles are bf16
            # anyway, and norm errors are row/column-structured, so norm
            # accuracy at bf16 level is provably negligible for the loss
            sqb = prep_pool.tile([8, NPTS], BF16)
            nxy = prep_pool.tile([2, NPTS], BF16)
            # full 8 rows (ACT needs 32-aligned partition bases); the src
            # rows squared here are dummies, overwritten from gts below
            nc.scalar.activation(sqb[:, :], pts[:, :], AF.Square)

            # transform: gts^T rows 0-2 (+ intact ones row 3)
            with tc.tile_pool(name="gts_ps", bufs=1, space="PSUM") as gpp:
                gts_ps = gpp.tile([4, NPTS], F32)
                for c in range(NPTS // 512):
                    cs = slice(c * 512, (c + 1) * 512)
                    nc.tensor.matmul(gts_ps[:, cs], lhsT=tlhs[:, :],
                                     rhs=trhs[:, cs], start=True, stop=True)
                nc.scalar.copy(pts[0:4, :], gts_ps[:, :])
                nc.scalar.activation(sqb[0:4, :], gts_ps[:, :], AF.Square)

            # bf16 hi/lo of (-2*gts | recon)
            c_hi = prep_pool.tile([8, NPTS], BF16)
            c_lo = prep_pool.tile([8, NPTS], BF16)
            nc.vector.tensor_scalar(c_hi[:, :], pts[:, :], scal[:, :], None,
                                    ALU.mult)
            nc.vector.scalar_tensor_tensor(c_lo[:, :], pts[:, :], scal[:, :],
                                           c_hi[:, :], ALU.mult, ALU.subtract)

            # xx (row 0) and yy (row 1) via one K=8 bf16 ones-matmul
            with tc.tile_pool(name="nrm_ps", bufs=1, space="PSUM") as npp:
                nrm_ps = npp.tile([2, NPTS], F32)
                for c in range(NPTS // 512):
                    cs = slice(c * 512, (c + 1) * 512)
                    nc.tensor.matmul(nrm_ps[:, cs], lhsT=norm_ones[:, :],
                                     rhs=sqb[:, cs], start=True, stop=True)
                nc.scalar.copy(nxy[:, :], nrm_ps[:, :])

            # assemble the K=11 matmul operands (SBUF->SBUF DMA row moves)
            # k 0-2: -2g_hi | p_hi   k 3-5: -2g_hi | p_lo   k 6-8: -2g_lo | p_hi
            # k 9:   1      | yy     k 10:  xx     | 1
            lhs = sb.tile([16, NPTS], BF16)
            rhs = sb.tile([16, NPTS], BF16)
            nc.sync.dma_start(out=lhs[0:3, :], in_=c_hi[0:3, :])
            nc.sync.dma_start(out=lhs[3:6, :], in_=c_hi[0:3, :])
            nc.sync.dma_start(out=lhs[6:9, :], in_=c_lo[0:3, :])
            nc.sync.dma_start(out=lhs[9:10, :], in_=c_hi[3:4, :])   # bf16 ones
            nc.sync.dma_start(out=lhs[10:11, :], in_=nxy[0:1, :])   # xx
            nc.sync.dma_start(out=rhs[0:3, :], in_=c_hi[4:7, :])
            nc.sync.dma_start(out=rhs[3:6, :], in_=c_lo[4:7, :])
            nc.sync.dma_start(out=rhs[6:9, :], in_=c_hi[4:7, :])
            nc.sync.dma_start(out=rhs[9:10, :], in_=nxy[1:2, :])    # yy
            nc.sync.dma_start(out=rhs[10:11, :], in_=c_hi[7:8, :])  # bf16 ones

            prep_pool.release()

            # ---- phase 3: distance tiles + min reductions ---------------
            rmin = sb.tile([128, NBLK], F32)        # per-block row mins
            mrun0 = sb.tile([128, NPTS], BF16)      # col-min over even blocks
            mrun1 = sb.tile([128, NPTS], BF16)      # col-min over odd blocks

            with tc.tile_pool(name="stage_sb", bufs=3) as stg, \
                 tc.tile_pool(name="main_ps", bufs=2, space="PSUM") as mps:
                for ip in range(NBLK // 2):
                    # stage a PAIR of row blocks, then one batched tree
                    pb = stg.tile([128, 2 * NPTS], BF16, tag="PSB", bufs=2)
                    for q in range(2):
                        ib = 2 * ip + q
                        lw = lhs[0:11, ib * 128:(ib + 1) * 128]
                        for h in range(2):
                            pt = mps.tile([128, HALF], F32, tag="P")
                            for s in range(HALF // 512):
                                j0 = h * HALF + s * 512
                                nc.tensor.matmul(
                                    pt[:, s * 512:(s + 1) * 512], lhsT=lw,
                                    rhs=rhs[0:11, j0:j0 + 512],
                                    start=True, stop=True)
                            # stage to bf16 SBUF (pure dtype-convert copy)
                            nc.scalar.copy(
                                pb[:, q * NPTS + h * HALF:
                                   q * NPTS + (h + 1) * HALF], pt[:, :])
                        # running col-min (dual accumulators so the two
                        # merge chains schedule independently on DVE)
                        pslice = pb[:, q * NPTS:(q + 1) * NPTS]
                        mr = mrun0 if q == 0 else mrun1
                        if ip == 0:
                            nc.vector.tensor_copy(mr[:, :], pslice)
                        else:
                            nc.vector.tensor_tensor(mr[:, :], pslice,
                                                    mr[:, :], ALU.min)
                    # batched row-min tree: [128, 2, w] views, both blocks
                    w = HALF
                    tr = pb.rearrange("p (b h w) -> p b h w", b=2, h=2)
                    lvl = 0
                    while w >= 128:
                        nt = stg.tile([128, 2, w], BF16, tag=f"TR{lvl}",
                                      bufs=2, name=f"tr{lvl}")
                        nc.vector.tensor_tensor(nt[:, :, :], tr[:, :, 0, :],
                                                tr[:, :, 1, :], ALU.min)
                        tr = nt.rearrange("p b (h w) -> p b h w", h=2)
                        w //= 2
                        lvl += 1
                    nc.vector.tensor_reduce(
                        rmin[:, 2 * ip:2 * ip + 2],
                        tr.rearrange("p b h w -> p b (h w)"), axis=AX.X,
                        op=ALU.min)

            # ---- phase 4: finishers -------------------------------------
            mrun = sb.tile([128, NPTS], BF16)
            rsum = sb.tile([128, 1], F32)
            cmin = sb.tile([128, NPTS // 128], F32)
            csum = sb.tile([128, 1], F32)
            tot = sb.tile([128, 1], F32)
            loss_sb = sb.tile([1, 1], F32)

            nc.vector.tensor_tensor(mrun[:, :], mrun0[:, :], mrun1[:, :],
                                    ALU.min)
            nc.vector.tensor_reduce(rsum[:, :], rmin[:, :], axis=AX.X,
                                    op=ALU.add)

            with tc.tile_pool(name="fin_ps", bufs=4, space="PSUM") as fps:
                # 8 transposes per PSUM tile, one batched fold per group
                for g in range(NPTS // 1024):
                    tp = fps.tile([128, 1024], BF16, tag="T")
                    for c in range(8):
                        j0 = (g * 8 + c) * 128
                        nc.tensor.transpose(tp[:, c * 128:(c + 1) * 128],
                                            mrun[:, j0:j0 + 128],
                                            ident_sb[:, :])
                    nc.vector.tensor_reduce(
                        cmin[:, 8 * g:8 * g + 8],
                        tp.rearrange("p (g w) -> p g w", w=128),
                        axis=AX.X, op=ALU.min)
                nc.vector.tensor_reduce(csum[:, :], cmin[:, :], axis=AX.X,
                                        op=ALU.add)
                nc.vector.tensor_tensor(tot[:, :], rsum[:, :], csum[:, :],
                                        ALU.add)

                loss_ps = fps.tile([1, 1], F32, tag="L", bufs=1)
                nc.tensor.matmul(loss_ps[:, :], lhsT=tot[:, :],
                                 rhs=ones128[:, :], start=True, stop=True)
                nc.scalar.copy(loss_sb[:, :], loss_ps[:, :])

            nc.sync.dma_start(out=loss[:, :], in_=loss_sb[:, :])

    nc.compile()
    return nc


def _prep_core_inputs(recon_b, src_b, transform_b):
    src_aug = np.empty((4, NPTS), np.float32)
    src_aug[0:3] = src_b.T
    src_aug[3] = 1.0
    rec_aug = np.empty((4, NPTS), np.float32)
    rec_aug[0:3] = recon_b.T
    rec_aug[3] = 1.0
    R = transform_b[:3, :3]
    t = transform_b[:3, 3]
    ta = np.zeros((4, 4), np.float32)
    ta[0:3, 0:3] = R.T
    ta[3, 0:3] = t
    ta[3, 3] = 1.0
    cnorm = np.zeros((8, 2), np.float32)
    cnorm[0:3, 0] = 1.0    # xx from gts squares
    cnorm[4:7, 1] = 1.0    # yy from recon squares
    cnorm = cnorm.astype(ml_dtypes.bfloat16)
    cscal = np.zeros((8, 1), np.float32)
    cscal[0:3] = -2.0
    cscal[3] = 1.0      # aug row -> bf16 ones source
    cscal[4:7] = 1.0
    cscal[7] = 1.0
    return {
        "srcT": np.ascontiguousarray(src_aug),
        "reconT": np.ascontiguousarray(rec_aug),
        "taug": ta,
        "ident": np.eye(128).astype(ml_dtypes.bfloat16),
        "cnorm": cnorm,
        "cscal": cscal,
        "cones": np.ones((128, 1), np.float32),
    }


def kernel(recon, src_points, transform):
    global LAST_RESULTS
    recon = np.asarray(recon, np.float32)
    src_points = np.asarray(src_points, np.float32)
    transform = np.asarray(transform, np.float32)
    B = recon.shape[0]
    assert B == N_CORES

    if "nc" not in _CACHE:
        _CACHE["nc"] = _build_kernel()
    nc = _CACHE["nc"]

    in_maps = [
        _prep_core_inputs(recon[b], src_points[b], transform[b])
        for b in range(B)
    ]
    res = run_bass_kernel_spmd(nc, in_maps, list(range(N_CORES)))
    LAST_RESULTS = res
    total = np.float64(0.0)
    for r in res.results:
        total += np.float64(r["loss"][0, 0])
    return np.float32(total)


# revision 10
# speedup vs baseline: 1.0034x; 1.0034x over previous
"""Chamfer loss kernel for Trainium2, batch-parallel over 8 NeuronCores.

Per core (one batch element b):
  gts = src_points[b] @ R^T + t          (on device, f32r matmul)
  P[i,j] = |gts_i|^2 + |recon_j|^2 - 2 gts_i . recon_j
  loss_b = sum_j min_i P + sum_i min_j P
Host sums the 8 partial losses.

Structure:
- All matmuls run in float32r (fp32 operands at bf16-rate streaming), so
  the distance matmul needs no hi/lo decomposition at all.  The host
  folds the -2 into the transform, so the device pipeline is just:
  transform -> squares -> norm row -> one augmented distance matmul.
- The augmented operands put xx / yy / ones on 32-aligned partition rows
  (k=0..3 coords+ones, k=32 xx|ones, k=64 ones|yy, everything between
  zeroed) because ACT copies must write at 32-aligned partition bases and
  extra K rows are free on the PE (cost is column-count bound).
- ACT stages each PSUM tile to bf16 SBUF (pure dtype copy, no bias).
- The staged bf16 tiles are reduced by DVE in 2x mode: two running
  elementwise-min accumulators for the per-column mins (even/odd blocks,
  so DVE isn't serialized on one dependency chain), and a batched binary
  min-tree (two row blocks at a time) for the per-row mins.
- Per-column mins are finished with PE transposes + free-axis folds, and
  everything is summed with a final ones-matmul across partitions.
"""

import os

# the axon client here has no NTFF profile hook; a stray BASS_TRACE=1 in the
# environment would crash run_bass_kernel_spmd on a missing import
os.environ["BASS_NEVER_TRACE"] = "1"

import ml_dtypes
import numpy as np

import concourse.bacc as bacc
import concourse.bass as bass
import concourse.mybir as mybir
import concourse.tile as tile
from concourse.bass_utils import run_bass_kernel_spmd

F32 = mybir.dt.float32
F32R = mybir.dt.float32r
BF16 = mybir.dt.bfloat16
ALU = mybir.AluOpType
AX = mybir.AxisListType
AF = mybir.ActivationFunctionType

N_CORES = 8
NPTS = 4096          # points per set (both gts and recon)
NBLK = NPTS // 128   # 32 row blocks
HALF = 2048          # P tile free width (4 PSUM banks)
KA = 65              # augmented operand rows (0-3, 32, 64 used)

_CACHE = {}
LAST_RESULTS = None


def _build_kernel():
    nc = bacc.Bacc("TRN2", target_bir_lowering=False, debug=False)

    srcT = nc.declare_dram_parameter("srcT", [4, NPTS], F32, isOutput=False)
    reconT = nc.declare_dram_parameter("reconT", [4, NPTS], F32, isOutput=False)
    taug = nc.declare_dram_parameter("taug", [4, 4], F32, isOutput=False)
    ident = nc.declare_dram_parameter("ident", [128, 128], BF16, isOutput=False)
    cnorm = nc.declare_dram_parameter("cnorm", [8, 2], F32, isOutput=False)
    cones = nc.declare_dram_parameter("cones", [128, 1], F32, isOutput=False)
    zeros = nc.declare_dram_parameter("zeros", [31, NPTS], F32, isOutput=False)
    loss = nc.declare_dram_parameter("loss", [1, 1], F32, isOutput=True)

    with tile.TileContext(nc) as tc:
        with tc.tile_pool(name="sb", bufs=1) as sb:
            prep_pool = tc.alloc_tile_pool(name="prep", bufs=1)
            # ---- phase 0: loads + operand-shell init --------------------
            pts = prep_pool.tile([8, NPTS], F32R) # 0-3 src_aug, 4-7 recon_aug
            for c in range(4):
                cs = slice(c * 1024, (c + 1) * 1024)
                nc.sync.dma_start(out=pts[0:4, cs], in_=srcT[:, cs].bitcast(F32R))
                nc.sync.dma_start(out=pts[4:8, cs], in_=reconT[:, cs].bitcast(F32R))

            taug_sb = sb.tile([4, 4], F32R)
            nc.sync.dma_start(out=taug_sb[:, :], in_=taug[:, :].bitcast(F32R))
            ident_sb = sb.tile([128, 128], BF16)
            nc.sync.dma_start(out=ident_sb[:, :], in_=ident[:, :])
            norm_ones = sb.tile([8, 2], F32R)
            nc.sync.dma_start(out=norm_ones[:, :], in_=cnorm[:, :].bitcast(F32R))
            ones128 = sb.tile([128, 1], F32)
            nc.sync.dma_start(out=ones128[:, :], in_=cones[:, :])

            # augmented distance-matmul operands; zero the unused K rows so
            # they contribute nothing (both sides zeroed: no 0*garbage NaNs)
            lhs = sb.tile([KA, NPTS], F32R)   # 0-2 -2g, 3 ones*, 32 xx, 64 one
            rhs = sb.tile([KA, NPTS], F32R)   # 0-2 p, 3 zero, 32 one, 64 yy
            zsrc = zeros[:, :].bitcast(F32R)
            nc.sync.dma_start(out=lhs[4:32, :], in_=zsrc[0:28, :])
            nc.sync.dma_start(out=lhs[33:64, :], in_=zsrc[0:31, :])
            nc.sync.dma_start(out=rhs[3:32, :], in_=zsrc[0:29, :])
            nc.sync.dma_start(out=rhs[33:64, :], in_=zsrc[0:31, :])

            # PE warm-up: tiny matmuls on the identity while inputs load,
            # so the transform/norm matmuls run at full PE clock
            with tc.tile_pool(name="warm_ps", bufs=1, space="PSUM") as wpp:
                warm_ps = wpp.tile([128, 128], F32)
                for _ in range(40):
                    nc.tensor.matmul(warm_ps[:, :], lhsT=ident_sb[:, :],
                                     rhs=ident_sb[:, :], start=True,
                                     stop=True)

            # recon side of the operands (no transform dependency): rows
            # 0-2 = p, ones rows on both sides (DMA is exempt from the
            # 32-aligned partition-base restriction)
            nc.sync.dma_start(out=rhs[0:3, :], in_=pts[4:7, :])
            nc.sync.dma_start(out=rhs[32:33, :], in_=pts[7:8, :])
            nc.sync.dma_start(out=lhs[64:65, :], in_=pts[7:8, :])

            # ---- phase 1: transform + norms -----------------------------
            # squares of all 8 rows early (gts rows are dummies for now);
            # the K=8 norm matmul's yy output row only weights rows 4-7,
            # so yy is valid before the transform lands
            sq = prep_pool.tile([8, NPTS], F32R)
            nc.scalar.activation(sq[:, :], pts[:, :], AF.Square)
            with tc.tile_pool(name="nrm_ps", bufs=1, space="PSUM") as npp:
                nrm_ps = npp.tile([2, NPTS], F32)
                for c in range(NPTS // 512):
                    cs = slice(c * 512, (c + 1) * 512)
                    nc.tensor.matmul(nrm_ps[:, cs], lhsT=norm_ones[:, :],
                                     rhs=sq[:, cs], start=True, stop=True)
                nc.scalar.copy(rhs[64:65, :], nrm_ps[0:1, :])   # yy

            # transform: rows 0-2 = -2*gts (host folded -2 into taug),
            # row 3 = ones; copied straight into the lhs operand
            with tc.tile_pool(name="gts_ps", bufs=1, space="PSUM") as gpp:
                gts_ps = gpp.tile([4, NPTS], F32)
                for c in range(NPTS // 512):
                    cs = slice(c * 512, (c + 1) * 512)
                    nc.tensor.matmul(gts_ps[:, cs], lhsT=taug_sb[:, :],
                                     rhs=pts[0:4, cs], start=True, stop=True)
                nc.scalar.copy(lhs[0:4, :], gts_ps[:, :])
                nc.scalar.activation(sq[0:4, :], gts_ps[:, :], AF.Square)

            # xx = 0.25 * sum((-2g)^2) via K=4 ones-matmul (weights 0.25)
            with tc.tile_pool(name="nrm2_ps", bufs=1, space="PSUM") as n2p:
                nrm2_ps = n2p.tile([1, NPTS], F32)
                for c in range(NPTS // 512):
                    cs = slice(c * 512, (c + 1) * 512)
                    nc.tensor.matmul(nrm2_ps[:, cs], lhsT=norm_ones[0:4, 1:2],
                                     rhs=sq[0:4, cs], start=True, stop=True)
                nc.scalar.copy(lhs[32:33, :], nrm2_ps[:, :])    # xx

            prep_pool.release()

            # ---- phase 3: distance tiles + min reductions ---------------
            rmin = sb.tile([128, NBLK], F32)        # per-block row mins
            mrun0 = sb.tile([128, NPTS], BF16)      # col-min over even blocks
            mrun1 = sb.tile([128, NPTS], BF16)      # col-min over odd blocks

            with tc.tile_pool(name="stage_sb", bufs=3) as stg, \
                 tc.tile_pool(name="main_ps", bufs=2, space="PSUM") as mps:
                for ip in range(NBLK // 2):
                    # stage a PAIR of row blocks, then one batched tree
                    pb = stg.tile([128, 2 * NPTS], BF16, tag="PSB", bufs=2)
                    for q in range(2):
                        ib = 2 * ip + q
                        lw = lhs[0:KA, ib * 128:(ib + 1) * 128]
                        for h in range(2):
                            pt = mps.tile([128, HALF], F32, tag="P")
                            for s in range(HALF // 512):
                                j0 = h * HALF + s * 512
                                nc.tensor.matmul(
                                    pt[:, s * 512:(s + 1) * 512], lhsT=lw,
                                    rhs=rhs[0:KA, j0:j0 + 512],
                                    start=True, stop=True)
                            # stage to bf16 SBUF (pure dtype-convert copy)
                            nc.scalar.copy(
                                pb[:, q * NPTS + h * HALF:
                                   q * NPTS + (h + 1) * HALF], pt[:, :])
                        # running col-min (dual accumulators so the two
                        # merge chains schedule independently on DVE)
                        pslice = pb[:, q * NPTS:(q + 1) * NPTS]
                        mr = mrun0 if q == 0 else mrun1
                        if ip == 0:
                            nc.vector.tensor_copy(mr[:, :], pslice)
                        else:
                            nc.vector.tensor_tensor(mr[:, :], pslice,
                                                    mr[:, :], ALU.min)
                    # batched row-min tree: [128, 2, w] views, both blocks
                    w = HALF
                    tr = pb.rearrange("p (b h w) -> p b h w", b=2, h=2)
                    lvl = 0
                    while w >= 128:
                        nt = stg.tile([128, 2, w], BF16, tag=f"TR{lvl}",
                                      bufs=2, name=f"tr{lvl}")
                        nc.vector.tensor_tensor(nt[:, :, :], tr[:, :, 0, :],
                                                tr[:, :, 1, :], ALU.min)
                        tr = nt.rearrange("p b (h w) -> p b h w", h=2)
                        w //= 2
                        lvl += 1
                    nc.vector.tensor_reduce(
                        rmin[:, 2 * ip:2 * ip + 2],
                        tr.rearrange("p b h w -> p b (h w)"), axis=AX.X,
                        op=ALU.min)

            # ---- phase 4: finishers -------------------------------------
            mrun = sb.tile([128, NPTS], BF16)
            rsum = sb.tile([128, 1], F32)
            cmin = sb.tile([128, NPTS // 128], F32)
            csum = sb.tile([128, 1], F32)
            tot = sb.tile([128, 1], F32)
            loss_sb = sb.tile([1, 1], F32)

            nc.vector.tensor_tensor(mrun[:, :], mrun0[:, :], mrun1[:, :],
                                    ALU.min)
            nc.vector.tensor_reduce(rsum[:, :], rmin[:, :], axis=AX.X,
                                    op=ALU.add)

            with tc.tile_pool(name="fin_ps", bufs=4, space="PSUM") as fps:
                # 8 transposes per PSUM tile, one batched fold per group
                for g in range(NPTS // 1024):
                    tp = fps.tile([128, 1024], BF16, tag="T")
                    for c in range(8):
                        j0 = (g * 8 + c) * 128
                        nc.tensor.transpose(tp[:, c * 128:(c + 1) * 128],
                                            mrun[:, j0:j0 + 128],
                                            ident_sb[:, :])
                    nc.vector.tensor_reduce(
                        cmin[:, 8 * g:8 * g + 8],
                        tp.rearrange("p (g w) -> p g w", w=128),
                        axis=AX.X, op=ALU.min)
                nc.vector.tensor_reduce(csum[:, :], cmin[:, :], axis=AX.X,
                                        op=ALU.add)
                nc.vector.tensor_tensor(tot[:, :], rsum[:, :], csum[:, :],
                                        ALU.add)

                loss_ps = fps.tile([1, 1], F32, tag="L", bufs=1)
                nc.tensor.matmul(loss_ps[:, :], lhsT=tot[:, :],
                                 rhs=ones128[:, :], start=True, stop=True)
                nc.scalar.copy(loss_sb[:, :], loss_ps[:, :])

            nc.sync.dma_start(out=loss[:, :], in_=loss_sb[:, :])

    nc.compile()
    return nc


def _prep_core_inputs(recon_b, src_b, transform_b):
    src_aug = np.empty((4, NPTS), np.float32)
    src_aug[0:3] = src_b.T
    src_aug[3] = 1.0
    rec_aug = np.empty((4, NPTS), np.float32)
    rec_aug[0:3] = recon_b.T
    rec_aug[3] = 1.0
    R = transform_b[:3, :3]
    t = transform_b[:3, 3]
    # -2 folded into the transform: device rows are -2*gts, and the xx
    # ones-matmul weights are 0.25 to undo the square of the -2
    ta = np.zeros((4, 4), np.float32)
    ta[0:3, 0:3] = -2.0 * R.T
    ta[3, 0:3] = -2.0 * t
    ta[3, 3] = 1.0
    cnorm = np.zeros((8, 2), np.float32)
    cnorm[4:7, 0] = 1.0    # col 0: yy from recon squares
    cnorm[0:3, 1] = 0.25   # col 1: xx from (-2*gts)^2 squares
    return {
        "srcT": np.ascontiguousarray(src_aug),
        "reconT": np.ascontiguousarray(rec_aug),
        "taug": ta,
        "ident": np.eye(128).astype(ml_dtypes.bfloat16),
        "cnorm": cnorm,
        "cones": np.ones((128, 1), np.float32),
        "zeros": np.zeros((31, NPTS), np.float32),
    }


def kernel(recon, src_points, transform):
    global LAST_RESULTS
    recon = np.asarray(recon, np.float32)
    src_points = np.asarray(src_points, np.float32)
    transform = np.asarray(transform, np.float32)
    B = recon.shape[0]
    assert B == N_CORES

    if "nc" not in _CACHE:
        _CACHE["nc"] = _build_kernel()
    nc = _CACHE["nc"]

    in_maps = [
        _prep_core_inputs(recon[b], src_points[b], transform[b])
        for b in range(B)
    ]
    res = run_bass_kernel_spmd(nc, in_maps, list(range(N_CORES)))
    LAST_RESULTS = res
    total = np.float64(0.0)
    for r in res.results:
        total += np.float64(r["loss"][0, 0])
    return np.float32(total)
